# revision 13
# baseline (speedup 1.0000x reference)
"""Trainium2 Bass kernel for a dense transformer decoder layer.

Shapes (hardcoded): B=2, S=2048, D=1024, H=16, HD=64, FF=4096, fp32 I/O.

Single fused launch, token-parallel over 8 cores (512 query tokens each;
batch b owned by cores 4b..4b+3). Each core redundantly computes K/V for
its batch's mask-compacted key tokens (host drops mask==0 keys and
zero-pads to a chunk multiple; zero K column -> harmless exp, zero
ones-column/V row -> no ctx or denominator contribution, so no mask bias
is needed anywhere). The ~12us of duplicated K/V GEMM work hides under
the softmax-exp ScalarE critical path and removes the second launch's
ramp/drain plus the K/V HBM round trip.

Precision: fp8e4m3 with DoubleRow matmuls for QKV projections, scores
operands, exp weights, V, and Wo; LN statistics, residual stream and
softmax normalization fp32; FFN in bf16 (fp8 exceeds the error budget).
"""

import os
import numpy as np
import ml_dtypes
from contextlib import ExitStack

import concourse.bass as bass
from concourse import bacc
import concourse.mybir as mybir
import concourse.tile as tile
from concourse.bass_utils import run_bass_kernel_spmd
from concourse.masks import make_identity

B, S, D, H, FF = 2, 2048, 1024, 16, 4096
HD = D // H
EPS = 1e-5
NCORES = 8
TOK = (B * S) // NCORES          # 512 query tokens per core
P = 128
DC = D // P                      # 8 contraction chunks
SBLK = TOK // P                  # 4 s-blocks of 128
FB = FF // P                     # 32 ff blocks of 128

F32 = mybir.dt.float32
BF16 = mybir.dt.bfloat16
FP8 = mybir.dt.float8e4
AF = mybir.ActivationFunctionType
ALU = mybir.AluOpType
DR = mybir.MatmulPerfMode.DoubleRow

bf16_np = ml_dtypes.bfloat16
fp8_np = ml_dtypes.float8_e4m3

T_PAD = 1152  # compacted key columns (only mask==1 keys kept, zero-padded)


def _ln_tile(nc, pools, x_tile, out_tile, eps_sb):
    """LayerNorm (no affine) of one [128, D] fp32 tile into out_tile."""
    stats = pools.tile([P, 2, 6], F32, tag="ln_stats")
    mv = pools.tile([P, 2], F32, tag="ln_mv")
    xg = x_tile.rearrange("p (g d) -> p g d", g=2)
    for g in range(2):
        nc.vector.bn_stats(out=stats[:, g, :], in_=xg[:, g, :])
    nc.vector.bn_aggr(out=mv[:], in_=stats[:])
    mean = mv[:, 0:1]
    std = pools.tile([P, 1], F32, tag="ln_std")
    nc.scalar.activation(out=std, in_=mv[:, 1:2], func=AF.Sqrt, bias=eps_sb, scale=1.0)
    nc.vector.reciprocal(out=std, in_=std)
    nc.vector.tensor_scalar(
        out=out_tile,
        in0=x_tile,
        scalar1=mean,
        scalar2=std,
        op0=ALU.subtract,
        op1=ALU.mult,
    )


def _build_fused(TKEY=T_PAD):
    NCH = TKEY // P                  # real key chunks (9 for 1152)
    NCHP = NCH + (NCH % 2)           # padded to even for DoubleRow (10)
    nc = bacc.Bacc(None, target_bir_lowering=False, debug=False)
    x_d = nc.declare_dram_parameter("x", [TOK, D], F32, isOutput=False)
    xk_d = nc.declare_dram_parameter("xk", [TKEY, D], F32, isOutput=False)
    oc_d = nc.declare_dram_parameter("oc", [P, NCHP], F32, isOutput=False)
    wq_d = nc.declare_dram_parameter("wq", [P, DC * D], FP8, isOutput=False)
    wk_d = nc.declare_dram_parameter("wk", [P, DC * D], FP8, isOutput=False)
    wv_d = nc.declare_dram_parameter("wv", [P, DC * D], FP8, isOutput=False)
    bq_d = nc.declare_dram_parameter("bq", [P, DC], F32, isOutput=False)
    bk_d = nc.declare_dram_parameter("bk", [P, DC], F32, isOutput=False)
    bv_d = nc.declare_dram_parameter("bvb", [P, D], F32, isOutput=False)
    wo_d = nc.declare_dram_parameter("wo", [P, DC * D], FP8, isOutput=False)
    bo_d = nc.declare_dram_parameter("bob", [P, D], F32, isOutput=False)
    w1_d = nc.declare_dram_parameter("w1", [D, FF], BF16, isOutput=False)
    b1_d = nc.declare_dram_parameter("b1", [P, FB], F32, isOutput=False)
    w2_d = nc.declare_dram_parameter("w2", [P, FB * D], BF16, isOutput=False)
    b2_d = nc.declare_dram_parameter("b2b", [P, D], F32, isOutput=False)
    out_d = nc.declare_dram_parameter("out", [TOK, D], F32, isOutput=True)

    with tile.TileContext(nc) as tc, ExitStack() as ctx:
        glob = ctx.enter_context(tc.tile_pool(name="glob", bufs=1))

        ident = glob.tile([P, P], BF16)
        make_identity(nc, ident)
        eps_sb = glob.tile([P, 1], F32)
        nc.vector.memset(eps_sb, EPS)
        ones_sb = glob.tile([1, 64], BF16)
        nc.vector.memset(ones_sb, 1.0)
        negone_sb = glob.tile([P, 1], F32)
        nc.vector.memset(negone_sb, -1.0)
        onesh_sb = glob.tile([P, H], F32)
        nc.vector.memset(onesh_sb, 1.0)

        qt_sb = glob.tile([P, DC, TOK], FP8)
        kt_sb = glob.tile([P, DC, TKEY], FP8)
        va_sb = glob.tile([P, NCHP, H, HD + 1], FP8)
        ctxu_sb = glob.tile([P, DC, TOK], BF16)      # unnormalized ctx^T
        ctxn_sb = glob.tile([P, DC, TOK], FP8)       # normalized ctx^T
        exp_bufs = [glob.tile([P, NCHP, 2, TOK], FP8, name=f"expb{i}")
                    for i in range(2)]
        if NCHP != NCH:
            for eb in exp_bufs:
                nc.gpsimd.memset(eb[:, NCH:NCHP, :, :], 0.0)
            nc.gpsimd.memset(va_sb[:, NCH:NCHP, :, :], 0.0)

        # residual-stream x tiles (also LN1 input) + misc small loads
        oc_sb = glob.tile([P, NCHP], F32)
        nc.sync.dma_start(out=oc_sb, in_=oc_d[:, :])
        x_tiles = []
        for sb in range(SBLK):
            xt = glob.tile([P, D], F32, name=f"x{sb}")
            nc.sync.dma_start(out=xt, in_=x_d[sb * P:(sb + 1) * P, :])
            x_tiles.append(xt)
        bo_sb = glob.tile([P, D], F32)
        b2_sb = glob.tile([P, D], F32)
        b1_sb = glob.tile([P, FB], F32)
        wo_sb = glob.tile([P, DC, D], FP8)
        resid_sb = glob.tile([P, SBLK, D], F32)
        z2t_sb = glob.tile([P, DC, TOK], BF16)

        with tc.tile_pool(name="qkv", bufs=1) as qkvp, \
             tc.tile_pool(name="lnw", bufs=3) as lnw, \
             tc.tile_pool(name="attn_sc", bufs=2, space="PSUM") as sc_psum, \
             tc.tile_pool(name="attn_cx", bufs=2, space="PSUM") as cx_psum, \
             tc.tile_pool(name="attn_wk", bufs=1) as awork:
            wq_sb = qkvp.tile([P, DC, D], FP8)
            wk_sb = qkvp.tile([P, DC, D], FP8)
            wv_sb = qkvp.tile([P, DC, D], FP8)
            bq_sb = qkvp.tile([P, DC], F32)
            bk_sb = qkvp.tile([P, DC], F32)
            bv_sb = qkvp.tile([P, D], F32)
            for dc in range(DC):
                nc.gpsimd.dma_start(out=wk_sb[:, dc, :],
                                    in_=wk_d[:, dc * D:(dc + 1) * D])
                nc.gpsimd.dma_start(out=wq_sb[:, dc, :],
                                    in_=wq_d[:, dc * D:(dc + 1) * D])
                nc.gpsimd.dma_start(out=wv_sb[:, dc, :],
                                    in_=wv_d[:, dc * D:(dc + 1) * D])
            nc.gpsimd.dma_start(out=bq_sb, in_=bq_d[:, :])
            nc.gpsimd.dma_start(out=bk_sb, in_=bk_d[:, :])
            nc.gpsimd.dma_start(out=bv_sb, in_=bv_d[:])
            nc.gpsimd.dma_start(out=bo_sb, in_=bo_d[:])
            nc.gpsimd.dma_start(out=b2_sb, in_=b2_d[:])
            nc.gpsimd.dma_start(out=b1_sb, in_=b1_d[:])
            nc.gpsimd.dma_start(
                out=wo_sb, in_=wo_d[:].rearrange("p (c n) -> p c n", c=DC))

            # LN1 of key tokens -> z1k^T, then own tokens -> z1^T
            z1kt_sb = qkvp.tile([P, DC, TKEY], FP8)
            z1t_sb = qkvp.tile([P, DC, TOK], FP8)
            for sb in range(NCH):
                xkt = lnw.tile([P, D], F32, tag="xk")
                nc.sync.dma_start(out=xkt, in_=xk_d[sb * P:(sb + 1) * P, :])
                z1k = lnw.tile([P, D], BF16, tag="z1k")
                _ln_tile(nc, lnw, xkt, z1k, eps_sb)
                for dc in range(DC):
                    pt = sc_psum.tile([P, P], BF16, tag="tp")
                    nc.tensor.transpose(pt, z1k[:, dc * P:(dc + 1) * P], ident)
                    nc.vector.tensor_copy(
                        out=z1kt_sb[:, dc, sb * P:(sb + 1) * P], in_=pt)
            for sb in range(SBLK):
                z1 = lnw.tile([P, D], BF16, tag="z1k")
                _ln_tile(nc, lnw, x_tiles[sb], z1, eps_sb)
                for dc in range(DC):
                    pt = sc_psum.tile([P, P], BF16, tag="tp")
                    nc.tensor.transpose(pt, z1[:, dc * P:(dc + 1) * P], ident)
                    nc.vector.tensor_copy(
                        out=z1t_sb[:, dc, sb * P:(sb + 1) * P], in_=pt)

            den8s = [awork.tile([8, TOK], F32, tag="den8", name=f"den8_{i}")
                     for i in range(2)]

            def kq_pair(pair):
                # kT columns for this pair, in <=512-col spans
                for c0 in range(0, TKEY, 512):
                    cw = min(512, TKEY - c0)
                    pk = cx_psum.tile([P, 512], F32, tag="cx")
                    for i in range(DC // 2):
                        nc.tensor.matmul(
                            pk[:, 0:cw],
                            lhsT=wk_sb[:, 2 * i:2 * i + 2, pair * P:(pair + 1) * P],
                            rhs=z1kt_sb[:, 2 * i:2 * i + 2, c0:c0 + cw],
                            start=(i == 0), stop=(i == DC // 2 - 1),
                            perf_mode=DR,
                        )
                    nc.vector.tensor_scalar_add(
                        out=kt_sb[:, pair, c0:c0 + cw], in0=pk[:, 0:cw],
                        scalar1=bk_sb[:, pair:pair + 1])
                pq = cx_psum.tile([P, 512], F32, tag="cx")
                for i in range(DC // 2):
                    nc.tensor.matmul(
                        pq,
                        lhsT=wq_sb[:, 2 * i:2 * i + 2, pair * P:(pair + 1) * P],
                        rhs=z1t_sb[:, 2 * i:2 * i + 2, :],
                        start=(i == 0), stop=(i == DC // 2 - 1),
                        perf_mode=DR,
                    )
                nc.vector.tensor_scalar_add(
                    out=qt_sb[:, pair, :], in0=pq,
                    scalar1=bq_sb[:, pair:pair + 1])

            def scores_pair(pair):
                et = exp_bufs[pair % 2]
                for tb in range(NCH):
                    ps = sc_psum.tile([P, 2, TOK], F32, tag="sc")
                    for hi in range(2):
                        po = 64 * hi
                        nc.tensor.matmul(
                            ps[:, hi, :],
                            lhsT=kt_sb[po:po + 64, pair, tb * P:(tb + 1) * P],
                            rhs=qt_sb[po:po + 64, pair, :],
                            start=True, stop=True,
                            tile_position=(po, 0),
                        )
                    nc.scalar.activation(
                        out=et[:, tb, :, :], in_=ps, func=AF.Exp,
                        bias=negone_sb, scale=1.0)

            def v_build():
                for sb in range(NCH):
                    vv = awork.tile([P, D], F32, tag="vv", name=f"vv{sb}")
                    for vh in range(2):
                        pv = cx_psum.tile([P, 512], F32, tag="cx")
                        for i in range(DC // 2):
                            nc.tensor.matmul(
                                pv,
                                lhsT=z1kt_sb[:, 2 * i:2 * i + 2,
                                             sb * P:(sb + 1) * P],
                                rhs=wv_sb[:, 2 * i:2 * i + 2,
                                          vh * 512:(vh + 1) * 512],
                                start=(i == 0), stop=(i == DC // 2 - 1),
                                perf_mode=DR,
                            )
                        nc.vector.tensor_add(
                            out=vv[:, vh * 512:(vh + 1) * 512], in0=pv,
                            in1=bv_sb[:, vh * 512:(vh + 1) * 512])
                    nc.vector.tensor_scalar(
                        out=va_sb[:, sb, :, 0:HD],
                        in0=vv.rearrange("p (h k) -> p h k", h=H),
                        scalar1=oc_sb[:, sb:sb + 1], scalar2=None,
                        op0=ALU.mult)
                    nc.vector.tensor_scalar(
                        out=va_sb[:, sb, :, HD:HD + 1],
                        in0=onesh_sb.rearrange("p (h o) -> p h o", o=1),
                        scalar1=oc_sb[:, sb:sb + 1], scalar2=None,
                        op0=ALU.mult)

            def ctx_pair(pair):
                et = exp_bufs[pair % 2]
                dstage = awork.tile([1, 2, TOK], F32, tag="dstage",
                                    name=f"dstage_{pair}")
                for hi in range(2):
                    h = pair * 2 + hi
                    pc = cx_psum.tile([HD + 1, TOK], F32, tag="cx")
                    for tg in range(NCHP // 2):
                        nc.tensor.matmul(
                            pc,
                            lhsT=va_sb[:, 2 * tg:2 * tg + 2, h, :],
                            rhs=et[:, 2 * tg:2 * tg + 2, hi, :],
                            start=(tg == 0), stop=(tg == NCHP // 2 - 1),
                            perf_mode=DR,
                        )
                    po = 64 * hi
                    nc.vector.tensor_copy(
                        out=ctxu_sb[po:po + 64, pair, :], in_=pc[0:HD, :])
                    nc.vector.tensor_copy(
                        out=dstage[:, hi, :], in_=pc[HD:HD + 1, :])
                j = pair % 4
                nc.sync.dma_start(
                    out=den8s[pair // 4][2 * j:2 * j + 2, :], in_=dstage)

            def norm_batch(bi):
                den8 = den8s[bi]
                den8r = awork.tile([8, TOK], F32, tag="den8r")
                dscr = awork.tile([8, TOK], F32, tag="dscr")
                nc.vector.reciprocal_approx_accurate(
                    out=den8r, in_=den8, scratch=dscr)
                rc8 = awork.tile([8, TOK], BF16, tag="rc8")
                nc.vector.tensor_copy(out=rc8, in_=den8r)
                rcflat = awork.tile([1, 8, TOK], BF16, tag="rcflat")
                nc.sync.dma_start(out=rcflat, in_=rc8)
                for j in range(4):
                    pj = 4 * bi + j
                    pb = cx_psum.tile([P, TOK], F32, tag="cx")
                    for hi in range(2):
                        po = 64 * hi
                        nc.tensor.matmul(
                            pb[po:po + 64, :], lhsT=ones_sb,
                            rhs=rcflat[:, 2 * j + hi, :],
                            start=True, stop=True,
                            tile_position=(0, po))
                    nc.vector.tensor_mul(
                        out=ctxn_sb[:, pj, :],
                        in0=ctxu_sb[:, pj, :], in1=pb)

            # software pipeline: scores of pair p+1 are emitted before
            # ctx of pair p so the ScalarE exp stream never starves
            kq_pair(0)
            scores_pair(0)
            for pair in range(1, DC):
                kq_pair(pair)
                scores_pair(pair)
                if pair == 1:
                    v_build()
                ctx_pair(pair - 1)
                if pair - 1 == 3:
                    norm_batch(0)
            ctx_pair(DC - 1)
            norm_batch(1)

        # --- Wo (fp8 DoubleRow) + residual + LN2 ---
        with tc.tile_pool(name="wo_psum", bufs=2, space="PSUM") as wo_psum, \
             tc.tile_pool(name="wo_wk", bufs=3) as wwork:
            for sb in range(SBLK):
                pw = wo_psum.tile([P, D], F32, tag="wo")
                for oh in range(2):
                    for i in range(DC // 2):
                        nc.tensor.matmul(
                            pw[:, oh * 512:(oh + 1) * 512],
                            lhsT=ctxn_sb[:, 2 * i:2 * i + 2, sb * P:(sb + 1) * P],
                            rhs=wo_sb[:, 2 * i:2 * i + 2, oh * 512:(oh + 1) * 512],
                            start=(i == 0), stop=(i == DC // 2 - 1),
                            perf_mode=DR,
                        )
                rs = resid_sb[:, sb, :]
                nc.vector.tensor_add(out=rs, in0=pw, in1=x_tiles[sb])
                nc.gpsimd.tensor_tensor(out=rs, in0=rs, in1=bo_sb, op=ALU.add)
                z2 = wwork.tile([P, D], BF16, tag="z2")
                _ln_tile(nc, wwork, rs, z2, eps_sb)
                for dc in range(DC):
                    pt = wo_psum.tile([P, P], BF16, tag="tp2")
                    nc.tensor.transpose(pt, z2[:, dc * P:(dc + 1) * P], ident)
                    nc.vector.tensor_copy(
                        out=z2t_sb[:, dc, sb * P:(sb + 1) * P], in_=pt)

        # --- FFN (bf16; fp8 fails the error budget) ---
        ffn = ctx.enter_context(tc.tile_pool(name="ffn", bufs=1))
        h1t_sb = ffn.tile([P, FB, TOK], BF16)
        with tc.tile_pool(name="ff_psum", bufs=2, space="PSUM") as fa_psum, \
             tc.tile_pool(name="ffb_psum", bufs=4, space="PSUM") as fb_psum, \
             tc.tile_pool(name="ff_w1", bufs=6) as w1pool, \
             tc.tile_pool(name="ff_w2", bufs=6) as w2pool, \
             tc.tile_pool(name="ff_wk", bufs=4) as fwork:
            po0_tiles = [fb_psum.tile([P, 512], F32, tag="ffb", name=f"po0_{sb}")
                         for sb in range(SBLK)]
            for fb in range(FB):
                w1t = w1pool.tile([P, DC, P], BF16, tag="w1t")
                nc.gpsimd.dma_start(
                    out=w1t,
                    in_=w1_d[:, fb * P:(fb + 1) * P].rearrange(
                        "(c p) f -> p c f", p=P))
                pf = fa_psum.tile([P, TOK], F32, tag="ffa")
                for dc in range(DC):
                    nc.tensor.matmul(
                        pf, lhsT=w1t[:, dc, :], rhs=z2t_sb[:, dc, :],
                        start=(dc == 0), stop=(dc == DC - 1))
                nc.scalar.activation(
                    out=h1t_sb[:, fb, :], in_=pf, func=AF.Relu,
                    bias=b1_sb[:, fb:fb + 1], scale=1.0)
                w2t = w2pool.tile([P, 512], BF16, tag="w2t")
                nc.gpsimd.dma_start(out=w2t, in_=w2_d[:, fb * D:fb * D + 512])
                for sb in range(SBLK):
                    nc.tensor.matmul(
                        po0_tiles[sb], lhsT=h1t_sb[:, fb, sb * P:(sb + 1) * P],
                        rhs=w2t,
                        start=(fb == 0), stop=(fb == FB - 1))
            for sb in range(SBLK):
                ot = fwork.tile([P, 512], F32, tag="out")
                nc.vector.tensor_add(out=ot, in0=po0_tiles[sb],
                                     in1=resid_sb[:, sb, 0:512])
                nc.gpsimd.tensor_tensor(out=ot, in0=ot, in1=b2_sb[:, 0:512],
                                        op=ALU.add)
                nc.sync.dma_start(out=out_d[sb * P:(sb + 1) * P, 0:512], in_=ot)
            po1_tiles = [fb_psum.tile([P, 512], F32, tag="ffb", name=f"po1_{sb}")
                         for sb in range(SBLK)]
            for fb in range(FB):
                w2t = w2pool.tile([P, 512], BF16, tag="w2t")
                nc.gpsimd.dma_start(
                    out=w2t, in_=w2_d[:, fb * D + 512:(fb + 1) * D])
                for sb in range(SBLK):
                    nc.tensor.matmul(
                        po1_tiles[sb], lhsT=h1t_sb[:, fb, sb * P:(sb + 1) * P],
                        rhs=w2t,
                        start=(fb == 0), stop=(fb == FB - 1))
            for sb in range(SBLK):
                ot = fwork.tile([P, 512], F32, tag="out")
                nc.vector.tensor_add(out=ot, in0=po1_tiles[sb],
                                     in1=resid_sb[:, sb, 512:1024])
                nc.gpsimd.tensor_tensor(out=ot, in0=ot, in1=b2_sb[:, 512:1024],
                                        op=ALU.add)
                nc.sync.dma_start(out=out_d[sb * P:(sb + 1) * P, 512:1024], in_=ot)

    return nc


_programs = {}
LAST_EXEC_NS = {}


def _get_program(tkey):
    if ("f", tkey) not in _programs:
        f = _build_fused(tkey)
        f.finalize()
        _programs[("f", tkey)] = f
    return _programs[("f", tkey)]


def kernel(**inputs):
    inp = {k: np.asarray(v) for k, v in inputs.items()}
    x = inp["x"].astype(np.float32).reshape(B * S, D)
    mask = inp["mask"].astype(np.int32)

    # ---- host-side weight prep (layout + LN-affine folding, fp32 math) ----
    scale = np.float32(1.0 / np.sqrt(HD))
    Wq = inp["Wq"].astype(np.float32).transpose(1, 0, 2).reshape(D, D)
    Wk = inp["Wk"].astype(np.float32).transpose(1, 0, 2).reshape(D, D)
    Wv = inp["Wv"].astype(np.float32).transpose(1, 0, 2).reshape(D, D)
    g1 = inp["ln1_g"].astype(np.float32)
    b1n = inp["ln1_b"].astype(np.float32)
    g2 = inp["ln2_g"].astype(np.float32)
    b2n = inp["ln2_b"].astype(np.float32)

    def chunk_part(w):  # [D, D] -> [P, DC*D] with row d = dc*128+p
        return np.ascontiguousarray(
            w.reshape(DC, P, D).transpose(1, 0, 2).reshape(P, DC * D))

    wq_p = chunk_part((g1[:, None] * Wq * scale).astype(fp8_np))
    bq_p = np.ascontiguousarray(
        ((b1n @ Wq) * scale + inp["bq"].astype(np.float32).reshape(-1) * scale)
        .reshape(DC, P).T).astype(np.float32)
    wk_p = chunk_part((g1[:, None] * Wk).astype(fp8_np))
    bk_p = np.ascontiguousarray(
        ((b1n @ Wk) + inp["bk"].astype(np.float32).reshape(-1))
        .reshape(DC, P).T).astype(np.float32)
    bv_p = ((b1n @ Wv) + inp["bv"].astype(np.float32).reshape(-1)).astype(np.float32)
    wv_p = chunk_part((g1[:, None] * Wv).astype(fp8_np))
    bv_b = np.ascontiguousarray(np.tile(bv_p[None, :], (P, 1)))

    wo_p = chunk_part(inp["Wo"].astype(np.float32).astype(fp8_np))
    bo_b = np.ascontiguousarray(
        np.tile(inp["bo"].astype(np.float32)[None, :], (P, 1)))
    w1_p = np.ascontiguousarray(
        (g2[:, None] * inp["W1"].astype(np.float32)).astype(bf16_np))
    b1_p = np.ascontiguousarray(
        ((b2n @ inp["W1"].astype(np.float32)) + inp["b1"].astype(np.float32))
        .reshape(FB, P).T).astype(np.float32)
    w2_p = np.ascontiguousarray(
        inp["W2"].astype(np.float32).astype(bf16_np)
        .reshape(FB, P, D).transpose(1, 0, 2).reshape(P, FB * D))
    b2_b = np.ascontiguousarray(
        np.tile(inp["b2"].astype(np.float32)[None, :], (P, 1)))

    counts = [int((mask[b] == 1).sum()) for b in range(B)]
    tkey = T_PAD if max(counts) <= T_PAD else ((max(counts) + P - 1) // P) * P
    nch = tkey // P
    nchp = nch + (nch % 2)
    prog = _get_program(tkey)
    core_ids = list(range(NCORES))
    profile = bool(os.environ.get("KERNEL_PROFILE"))
    kw = {"trace": True} if profile else {}

    # per-batch compacted key tokens + ones-column layout [P, NCHP]
    xk_b, oc_b = [], []
    for b in range(B):
        idx = np.nonzero(mask[b] == 1)[0]
        n = len(idx)
        xk = np.zeros((tkey, D), np.float32)
        xk[:n] = x[b * S:(b + 1) * S][idx]
        oc = np.zeros((nch * P,), np.float32)
        oc[:n] = 1.0
        ocp = np.zeros((P, nchp), np.float32)
        ocp[:, :nch] = oc.reshape(nch, P).T
        xk_b.append(np.ascontiguousarray(xk))
        oc_b.append(np.ascontiguousarray(ocp))

    in_maps = []
    for c in range(NCORES):
        b = c // 4
        in_maps.append({
            "x": np.ascontiguousarray(x[c * TOK:(c + 1) * TOK, :]),
            "xk": xk_b[b], "oc": oc_b[b],
            "wq": wq_p, "wk": wk_p, "wv": wv_p,
            "bq": bq_p, "bk": bk_p, "bvb": bv_b,
            "wo": wo_p, "bob": bo_b,
            "w1": w1_p, "b1": b1_p, "w2": w2_p, "b2b": b2_b,
        })
    r = run_bass_kernel_spmd(prog, in_maps, core_ids, **kw)

    if profile:
        LAST_EXEC_NS.clear()
        LAST_EXEC_NS["l1"] = 0
        LAST_EXEC_NS["l2"] = r.exec_time_ns
        LAST_EXEC_NS["l2_trace"] = getattr(r, "instructions_and_trace", None)

    out = np.concatenate([r.results[c]["out"] for c in range(NCORES)], axis=0)
    return out.reshape(B, S, D).astype(np.float32)


# revision 21
# speedup vs baseline: 1.0712x; 1.0712x over previous
"""Trainium2 Bass kernel for a dense transformer decoder layer.

Shapes (hardcoded): B=2, S=2048, D=1024, H=16, HD=64, FF=4096, fp32 I/O.

Single fused launch, token-parallel over 8 cores (512 query tokens each;
batch b owned by cores 4b..4b+3). Each core redundantly computes K/V for
its batch's mask-compacted key tokens (host drops mask==0 keys and
zero-pads to a chunk multiple; zero K column -> harmless exp, zero
ones-column/V row -> no ctx or denominator contribution, so no mask bias
is needed anywhere). The ~12us of duplicated K/V GEMM work hides under
the softmax-exp ScalarE critical path and removes the second launch's
ramp/drain plus the K/V HBM round trip.

Precision: fp8e4m3 with DoubleRow matmuls for QKV projections, scores
operands, exp weights, V, and Wo; LN statistics, residual stream and
softmax normalization fp32; FFN in bf16 (fp8 exceeds the error budget).
"""

import os
import numpy as np
import ml_dtypes
from contextlib import ExitStack

import concourse.bass as bass
from concourse import bacc
import concourse.mybir as mybir
import concourse.tile as tile
from concourse.bass_utils import run_bass_kernel_spmd
from concourse.masks import make_identity

B, S, D, H, FF = 2, 2048, 1024, 16, 4096
HD = D // H
EPS = 1e-5
NCORES = 8
TOK = (B * S) // NCORES          # 512 query tokens per core
P = 128
DC = D // P                      # 8 contraction chunks
SBLK = TOK // P                  # 4 s-blocks of 128
FB = FF // P                     # 32 ff blocks of 128

F32 = mybir.dt.float32
BF16 = mybir.dt.bfloat16
FP8 = mybir.dt.float8e4
AF = mybir.ActivationFunctionType
ALU = mybir.AluOpType
DR = mybir.MatmulPerfMode.DoubleRow

bf16_np = ml_dtypes.bfloat16
fp8_np = ml_dtypes.float8_e4m3

T_PAD = 1152  # compacted key columns (only mask==1 keys kept, zero-padded)


def _ln_tile(nc, pools, x_tile, out_tile, eps_sb):
    """LayerNorm (no affine) of one [128, D] fp32 tile into out_tile."""
    stats = pools.tile([P, 2, 6], F32, tag="ln_stats")
    mv = pools.tile([P, 2], F32, tag="ln_mv")
    xg = x_tile.rearrange("p (g d) -> p g d", g=2)
    for g in range(2):
        nc.vector.bn_stats(out=stats[:, g, :], in_=xg[:, g, :])
    nc.vector.bn_aggr(out=mv[:], in_=stats[:])
    mean = mv[:, 0:1]
    std = pools.tile([P, 1], F32, tag="ln_std")
    nc.scalar.activation(out=std, in_=mv[:, 1:2], func=AF.Sqrt, bias=eps_sb, scale=1.0)
    nc.vector.reciprocal(out=std, in_=std)
    nc.vector.tensor_scalar(
        out=out_tile,
        in0=x_tile,
        scalar1=mean,
        scalar2=std,
        op0=ALU.subtract,
        op1=ALU.mult,
    )


def _build_fused(TKEY=T_PAD):
    NCH = TKEY // P                  # real key chunks (9 for 1152)
    NCHP = NCH + (NCH % 2)           # padded to even for DoubleRow (10)
    nc = bacc.Bacc(None, target_bir_lowering=False, debug=False)
    x_d = nc.declare_dram_parameter("x", [TOK, D], F32, isOutput=False)
    xk_d = nc.declare_dram_parameter("xk", [TKEY, D], BF16, isOutput=False)
    oc_d = nc.declare_dram_parameter("oc", [P, NCHP], F32, isOutput=False)
    wq_d = nc.declare_dram_parameter("wq", [P, DC * D], FP8, isOutput=False)
    wk_d = nc.declare_dram_parameter("wk", [P, DC * D], FP8, isOutput=False)
    wv_d = nc.declare_dram_parameter("wv", [P, DC * D], FP8, isOutput=False)
    bq_d = nc.declare_dram_parameter("bq", [P, DC], F32, isOutput=False)
    bk_d = nc.declare_dram_parameter("bk", [P, DC], F32, isOutput=False)
    bv_d = nc.declare_dram_parameter("bvb", [P, D], F32, isOutput=False)
    wo_d = nc.declare_dram_parameter("wo", [P, DC * D], FP8, isOutput=False)
    bo_d = nc.declare_dram_parameter("bob", [P, D], F32, isOutput=False)
    w1_d = nc.declare_dram_parameter("w1", [D, FF], BF16, isOutput=False)
    b1_d = nc.declare_dram_parameter("b1", [P, FB], F32, isOutput=False)
    w2_d = nc.declare_dram_parameter("w2", [P, FB * D], BF16, isOutput=False)
    b2_d = nc.declare_dram_parameter("b2b", [P, D], F32, isOutput=False)
    out_d = nc.declare_dram_parameter("out", [TOK, D], F32, isOutput=True)

    with tile.TileContext(nc) as tc, ExitStack() as ctx:
        glob = ctx.enter_context(tc.tile_pool(name="glob", bufs=1))

        ident = glob.tile([P, P], BF16)
        make_identity(nc, ident)
        eps_sb = glob.tile([P, 1], F32)
        nc.vector.memset(eps_sb, EPS)
        ones_sb = glob.tile([1, 64], BF16)
        nc.vector.memset(ones_sb, 1.0)
        negone_sb = glob.tile([P, 1], F32)
        nc.vector.memset(negone_sb, -1.0)
        onesh_sb = glob.tile([P, H], F32)
        nc.vector.memset(onesh_sb, 1.0)

        qt_sb = glob.tile([P, DC, TOK], FP8)
        kt_sb = glob.tile([P, DC, TKEY], FP8)
        va_sb = glob.tile([P, NCHP, H, HD + 1], FP8)
        ctxu_sb = glob.tile([P, DC, TOK], BF16)      # unnormalized ctx^T
        ctxn_sb = glob.tile([P, DC, TOK], FP8)       # normalized ctx^T
        exp_bufs = [glob.tile([P, NCHP, 2, TOK], FP8, name=f"expb{i}")
                    for i in range(2)]
        if NCHP != NCH:
            for eb in exp_bufs:
                nc.gpsimd.memset(eb[:, NCH:NCHP, :, :], 0.0)
            nc.gpsimd.memset(va_sb[:, NCH:NCHP, :, :], 0.0)

        # residual-stream x tiles (also LN1 input) + misc small loads;
        # key-token tiles go first on the sync queue (critical path)
        xk_tiles = []
        for sb in range(TKEY // P):
            xkt = glob.tile([P, D], BF16, name=f"xk{sb}")
            nc.sync.dma_start(out=xkt, in_=xk_d[sb * P:(sb + 1) * P, :])
            xk_tiles.append(xkt)
        oc_sb = glob.tile([P, NCHP], F32)
        nc.sync.dma_start(out=oc_sb, in_=oc_d[:, :])
        x_tiles = []
        for sb in range(SBLK):
            xt = glob.tile([P, D], F32, name=f"x{sb}")
            nc.sync.dma_start(out=xt, in_=x_d[sb * P:(sb + 1) * P, :])
            x_tiles.append(xt)
        bo_sb = glob.tile([P, D], F32)
        b2_sb = glob.tile([P, D], F32)
        b1_sb = glob.tile([P, FB], F32)
        wo_sb = glob.tile([P, DC, D], FP8)
        resid_sb = glob.tile([P, SBLK, D], F32)
        z2t_sb = glob.tile([P, DC, TOK], BF16)

        with tc.tile_pool(name="qkv", bufs=1) as qkvp, \
             tc.tile_pool(name="lnw", bufs=3) as lnw, \
             tc.tile_pool(name="attn_sc", bufs=3, space="PSUM") as sc_psum, \
             tc.tile_pool(name="attn_cx", bufs=2, space="PSUM") as cx_psum, \
             tc.tile_pool(name="attn_wk", bufs=1) as awork:
            wq_sb = qkvp.tile([P, DC, D], FP8)
            wk_sb = qkvp.tile([P, DC, D], FP8)
            wv_sb = qkvp.tile([P, DC, D], FP8)
            bq_sb = qkvp.tile([P, DC], F32)
            bk_sb = qkvp.tile([P, DC], F32)
            bv_sb = qkvp.tile([P, D], F32)
            for dc in range(DC):
                nc.gpsimd.dma_start(out=wk_sb[:, dc, :],
                                    in_=wk_d[:, dc * D:(dc + 1) * D])
                nc.gpsimd.dma_start(out=wq_sb[:, dc, :],
                                    in_=wq_d[:, dc * D:(dc + 1) * D])
                nc.gpsimd.dma_start(out=wv_sb[:, dc, :],
                                    in_=wv_d[:, dc * D:(dc + 1) * D])
            nc.gpsimd.dma_start(out=bq_sb, in_=bq_d[:, :])
            nc.gpsimd.dma_start(out=bk_sb, in_=bk_d[:, :])
            nc.gpsimd.dma_start(out=bv_sb, in_=bv_d[:])
            nc.gpsimd.dma_start(out=bo_sb, in_=bo_d[:])
            nc.gpsimd.dma_start(out=b2_sb, in_=b2_d[:])
            nc.gpsimd.dma_start(out=b1_sb, in_=b1_d[:])
            nc.gpsimd.dma_start(
                out=wo_sb, in_=wo_d[:].rearrange("p (c n) -> p c n", c=DC))

            # LN1 of key tokens -> z1k^T, then own tokens -> z1^T; the
            # PSUM->SBUF copies round-robin over 3 engines so the DVE
            # (busy with LN stats) isn't the serial bottleneck
            z1kt_sb = qkvp.tile([P, DC, TKEY], FP8)
            z1t_sb = qkvp.tile([P, DC, TOK], FP8)
            cpeng = [nc.vector, nc.scalar, nc.vector]

            def ln_t(src, dst_sb, dst_col, ci):
                z1k = lnw.tile([P, D], BF16, tag="z1k")
                _ln_tile(nc, lnw, src, z1k, eps_sb)
                for dc in range(DC):
                    pt = sc_psum.tile([P, P], BF16, tag="sc")
                    nc.tensor.transpose(pt, z1k[:, dc * P:(dc + 1) * P], ident)
                    eng = cpeng[(ci + dc) % 3]
                    if eng is nc.scalar:
                        eng.copy(out=dst_sb[:, dc, dst_col:dst_col + P], in_=pt)
                    else:
                        eng.tensor_copy(
                            out=dst_sb[:, dc, dst_col:dst_col + P], in_=pt)

            for sb in range(NCH):
                ln_t(xk_tiles[sb], z1kt_sb, sb * P, sb)
            for sb in range(SBLK):
                ln_t(x_tiles[sb], z1t_sb, sb * P, sb)

            den8s = [awork.tile([8, TOK], F32, tag="den8", name=f"den8_{i}")
                     for i in range(2)]

            def kq_pair(pair):
                # kT columns for this pair, in <=512-col spans
                for c0 in range(0, TKEY, 512):
                    cw = min(512, TKEY - c0)
                    pk = cx_psum.tile([P, 512], F32, tag="cx")
                    for i in range(DC // 2):
                        nc.tensor.matmul(
                            pk[:, 0:cw],
                            lhsT=wk_sb[:, 2 * i:2 * i + 2, pair * P:(pair + 1) * P],
                            rhs=z1kt_sb[:, 2 * i:2 * i + 2, c0:c0 + cw],
                            start=(i == 0), stop=(i == DC // 2 - 1),
                            perf_mode=DR,
                        )
                    nc.vector.tensor_scalar_add(
                        out=kt_sb[:, pair, c0:c0 + cw], in0=pk[:, 0:cw],
                        scalar1=bk_sb[:, pair:pair + 1])
                pq = cx_psum.tile([P, 512], F32, tag="cx")
                for i in range(DC // 2):
                    nc.tensor.matmul(
                        pq,
                        lhsT=wq_sb[:, 2 * i:2 * i + 2, pair * P:(pair + 1) * P],
                        rhs=z1t_sb[:, 2 * i:2 * i + 2, :],
                        start=(i == 0), stop=(i == DC // 2 - 1),
                        perf_mode=DR,
                    )
                nc.vector.tensor_scalar_add(
                    out=qt_sb[:, pair, :], in0=pq,
                    scalar1=bq_sb[:, pair:pair + 1])

            def scores_pair(pair):
                et = exp_bufs[pair % 2]
                for tb in range(NCH):
                    ps = sc_psum.tile([P, 2, TOK], F32, tag="sc")
                    for hi in range(2):
                        po = 64 * hi
                        nc.tensor.matmul(
                            ps[:, hi, :],
                            lhsT=kt_sb[po:po + 64, pair, tb * P:(tb + 1) * P],
                            rhs=qt_sb[po:po + 64, pair, :],
                            start=True, stop=True,
                            tile_position=(po, 0),
                        )
                    nc.scalar.activation(
                        out=et[:, tb, :, :], in_=ps, func=AF.Exp,
                        bias=negone_sb, scale=1.0)

            def v_build():
                for sb in range(NCH):
                    vv = awork.tile([P, D], F32, tag="vv", name=f"vv{sb}")
                    for vh in range(2):
                        pv = cx_psum.tile([P, 512], F32, tag="cx")
                        for i in range(DC // 2):
                            nc.tensor.matmul(
                                pv,
                                lhsT=z1kt_sb[:, 2 * i:2 * i + 2,
                                             sb * P:(sb + 1) * P],
                                rhs=wv_sb[:, 2 * i:2 * i + 2,
                                          vh * 512:(vh + 1) * 512],
                                start=(i == 0), stop=(i == DC // 2 - 1),
                                perf_mode=DR,
                            )
                        nc.vector.tensor_add(
                            out=vv[:, vh * 512:(vh + 1) * 512], in0=pv,
                            in1=bv_sb[:, vh * 512:(vh + 1) * 512])
                    nc.vector.tensor_scalar(
                        out=va_sb[:, sb, :, 0:HD],
                        in0=vv.rearrange("p (h k) -> p h k", h=H),
                        scalar1=oc_sb[:, sb:sb + 1], scalar2=None,
                        op0=ALU.mult)
                    nc.vector.tensor_scalar(
                        out=va_sb[:, sb, :, HD:HD + 1],
                        in0=onesh_sb.rearrange("p (h o) -> p h o", o=1),
                        scalar1=oc_sb[:, sb:sb + 1], scalar2=None,
                        op0=ALU.mult)

            def ctx_pair(pair):
                et = exp_bufs[pair % 2]
                dstage = awork.tile([1, 2, TOK], F32, tag="dstage",
                                    name=f"dstage_{pair}")
                for hi in range(2):
                    h = pair * 2 + hi
                    pc = cx_psum.tile([HD + 1, TOK], F32, tag="cx")
                    for tg in range(NCHP // 2):
                        nc.tensor.matmul(
                            pc,
                            lhsT=va_sb[:, 2 * tg:2 * tg + 2, h, :],
                            rhs=et[:, 2 * tg:2 * tg + 2, hi, :],
                            start=(tg == 0), stop=(tg == NCHP // 2 - 1),
                            perf_mode=DR,
                        )
                    po = 64 * hi
                    nc.vector.tensor_copy(
                        out=ctxu_sb[po:po + 64, pair, :], in_=pc[0:HD, :])
                    nc.vector.tensor_copy(
                        out=dstage[:, hi, :], in_=pc[HD:HD + 1, :])
                j = pair % 4
                nc.sync.dma_start(
                    out=den8s[pair // 4][2 * j:2 * j + 2, :], in_=dstage)

            def norm_batch(bi):
                den8 = den8s[bi]
                den8r = awork.tile([8, TOK], F32, tag="den8r")
                dscr = awork.tile([8, TOK], F32, tag="dscr")
                nc.vector.reciprocal_approx_accurate(
                    out=den8r, in_=den8, scratch=dscr)
                rc8 = awork.tile([8, TOK], BF16, tag="rc8")
                nc.vector.tensor_copy(out=rc8, in_=den8r)
                rcflat = awork.tile([1, 8, TOK], BF16, tag="rcflat")
                nc.sync.dma_start(out=rcflat, in_=rc8)
                for j in range(4):
                    pj = 4 * bi + j
                    pb = cx_psum.tile([P, TOK], F32, tag="cx")
                    for hi in range(2):
                        po = 64 * hi
                        nc.tensor.matmul(
                            pb[po:po + 64, :], lhsT=ones_sb,
                            rhs=rcflat[:, 2 * j + hi, :],
                            start=True, stop=True,
                            tile_position=(0, po))
                    nc.vector.tensor_mul(
                        out=ctxn_sb[:, pj, :],
                        in0=ctxu_sb[:, pj, :], in1=pb)

            # software pipeline: scores of pair p+1 are emitted before
            # ctx of pair p so the ScalarE exp stream never starves
            kq_pair(0)
            scores_pair(0)
            for pair in range(1, DC):
                kq_pair(pair)
                scores_pair(pair)
                if pair == 1:
                    v_build()
                ctx_pair(pair - 1)
                if pair == 6:
                    norm_batch(0)
            ctx_pair(DC - 1)
            norm_batch(1)

        # --- Wo (fp8 DoubleRow) + residual + LN2 ---
        with tc.tile_pool(name="wo_psum", bufs=2, space="PSUM") as wo_psum, \
             tc.tile_pool(name="wo_wk", bufs=3) as wwork:
            for sb in range(SBLK):
                pw = wo_psum.tile([P, D], F32, tag="wo")
                for oh in range(2):
                    for i in range(DC // 2):
                        nc.tensor.matmul(
                            pw[:, oh * 512:(oh + 1) * 512],
                            lhsT=ctxn_sb[:, 2 * i:2 * i + 2, sb * P:(sb + 1) * P],
                            rhs=wo_sb[:, 2 * i:2 * i + 2, oh * 512:(oh + 1) * 512],
                            start=(i == 0), stop=(i == DC // 2 - 1),
                            perf_mode=DR,
                        )
                rs = resid_sb[:, sb, :]
                nc.vector.tensor_add(out=rs, in0=pw, in1=x_tiles[sb])
                nc.gpsimd.tensor_tensor(out=rs, in0=rs, in1=bo_sb, op=ALU.add)
                z2 = wwork.tile([P, D], BF16, tag="z2")
                _ln_tile(nc, wwork, rs, z2, eps_sb)
                for dc in range(DC):
                    pt = wo_psum.tile([P, P], BF16, tag="tp2")
                    nc.tensor.transpose(pt, z2[:, dc * P:(dc + 1) * P], ident)
                    nc.vector.tensor_copy(
                        out=z2t_sb[:, dc, sb * P:(sb + 1) * P], in_=pt)

        # --- FFN (bf16; fp8 fails the error budget) ---
        ffn = ctx.enter_context(tc.tile_pool(name="ffn", bufs=1))
        h1t_sb = ffn.tile([P, FB, TOK], BF16)
        with tc.tile_pool(name="ff_psum", bufs=2, space="PSUM") as fa_psum, \
             tc.tile_pool(name="ffb_psum", bufs=4, space="PSUM") as fb_psum, \
             tc.tile_pool(name="ff_w1", bufs=6) as w1pool, \
             tc.tile_pool(name="ff_w2", bufs=6) as w2pool, \
             tc.tile_pool(name="ff_wk", bufs=4) as fwork:
            po0_tiles = [fb_psum.tile([P, 512], F32, tag="ffb", name=f"po0_{sb}")
                         for sb in range(SBLK)]
            for fb in range(FB):
                w1t = w1pool.tile([P, DC, P], BF16, tag="w1t")
                nc.gpsimd.dma_start(
                    out=w1t,
                    in_=w1_d[:, fb * P:(fb + 1) * P].rearrange(
                        "(c p) f -> p c f", p=P))
                pf = fa_psum.tile([P, TOK], F32, tag="ffa")
                for dc in range(DC):
                    nc.tensor.matmul(
                        pf, lhsT=w1t[:, dc, :], rhs=z2t_sb[:, dc, :],
                        start=(dc == 0), stop=(dc == DC - 1))
                nc.scalar.activation(
                    out=h1t_sb[:, fb, :], in_=pf, func=AF.Relu,
                    bias=b1_sb[:, fb:fb + 1], scale=1.0)
                w2t = w2pool.tile([P, 512], BF16, tag="w2t")
                nc.gpsimd.dma_start(out=w2t, in_=w2_d[:, fb * D:fb * D + 512])
                for sb in range(SBLK):
                    nc.tensor.matmul(
                        po0_tiles[sb], lhsT=h1t_sb[:, fb, sb * P:(sb + 1) * P],
                        rhs=w2t,
                        start=(fb == 0), stop=(fb == FB - 1))
            for sb in range(SBLK):
                ot = fwork.tile([P, 512], F32, tag="out")
                nc.vector.tensor_add(out=ot, in0=po0_tiles[sb],
                                     in1=resid_sb[:, sb, 0:512])
                nc.gpsimd.tensor_tensor(out=ot, in0=ot, in1=b2_sb[:, 0:512],
                                        op=ALU.add)
                nc.sync.dma_start(out=out_d[sb * P:(sb + 1) * P, 0:512], in_=ot)
            po1_tiles = [fb_psum.tile([P, 512], F32, tag="ffb", name=f"po1_{sb}")
                         for sb in range(SBLK)]
            for fb in range(FB):
                w2t = w2pool.tile([P, 512], BF16, tag="w2t")
                nc.gpsimd.dma_start(
                    out=w2t, in_=w2_d[:, fb * D + 512:(fb + 1) * D])
                for sb in range(SBLK):
                    nc.tensor.matmul(
                        po1_tiles[sb], lhsT=h1t_sb[:, fb, sb * P:(sb + 1) * P],
                        rhs=w2t,
                        start=(fb == 0), stop=(fb == FB - 1))
            for sb in range(SBLK):
                ot = fwork.tile([P, 512], F32, tag="out")
                nc.vector.tensor_add(out=ot, in0=po1_tiles[sb],
                                     in1=resid_sb[:, sb, 512:1024])
                nc.gpsimd.tensor_tensor(out=ot, in0=ot, in1=b2_sb[:, 512:1024],
                                        op=ALU.add)
                nc.sync.dma_start(out=out_d[sb * P:(sb + 1) * P, 512:1024], in_=ot)

    return nc


_programs = {}
LAST_EXEC_NS = {}


def _get_program(tkey):
    if ("f", tkey) not in _programs:
        f = _build_fused(tkey)
        f.finalize()
        _programs[("f", tkey)] = f
    return _programs[("f", tkey)]


def kernel(**inputs):
    inp = {k: np.asarray(v) for k, v in inputs.items()}
    x = inp["x"].astype(np.float32).reshape(B * S, D)
    mask = inp["mask"].astype(np.int32)

    # ---- host-side weight prep (layout + LN-affine folding, fp32 math) ----
    scale = np.float32(1.0 / np.sqrt(HD))
    Wq = inp["Wq"].astype(np.float32).transpose(1, 0, 2).reshape(D, D)
    Wk = inp["Wk"].astype(np.float32).transpose(1, 0, 2).reshape(D, D)
    Wv = inp["Wv"].astype(np.float32).transpose(1, 0, 2).reshape(D, D)
    g1 = inp["ln1_g"].astype(np.float32)
    b1n = inp["ln1_b"].astype(np.float32)
    g2 = inp["ln2_g"].astype(np.float32)
    b2n = inp["ln2_b"].astype(np.float32)

    def chunk_part(w):  # [D, D] -> [P, DC*D] with row d = dc*128+p
        return np.ascontiguousarray(
            w.reshape(DC, P, D).transpose(1, 0, 2).reshape(P, DC * D))

    wq_p = chunk_part((g1[:, None] * Wq * scale).astype(fp8_np))
    bq_p = np.ascontiguousarray(
        ((b1n @ Wq) * scale + inp["bq"].astype(np.float32).reshape(-1) * scale)
        .reshape(DC, P).T).astype(np.float32)
    wk_p = chunk_part((g1[:, None] * Wk).astype(fp8_np))
    bk_p = np.ascontiguousarray(
        ((b1n @ Wk) + inp["bk"].astype(np.float32).reshape(-1))
        .reshape(DC, P).T).astype(np.float32)
    bv_p = ((b1n @ Wv) + inp["bv"].astype(np.float32).reshape(-1)).astype(np.float32)
    wv_p = chunk_part((g1[:, None] * Wv).astype(fp8_np))
    bv_b = np.ascontiguousarray(np.tile(bv_p[None, :], (P, 1)))

    wo_p = chunk_part(inp["Wo"].astype(np.float32).astype(fp8_np))
    bo_b = np.ascontiguousarray(
        np.tile(inp["bo"].astype(np.float32)[None, :], (P, 1)))
    w1_p = np.ascontiguousarray(
        (g2[:, None] * inp["W1"].astype(np.float32)).astype(bf16_np))
    b1_p = np.ascontiguousarray(
        ((b2n @ inp["W1"].astype(np.float32)) + inp["b1"].astype(np.float32))
        .reshape(FB, P).T).astype(np.float32)
    w2_p = np.ascontiguousarray(
        inp["W2"].astype(np.float32).astype(bf16_np)
        .reshape(FB, P, D).transpose(1, 0, 2).reshape(P, FB * D))
    b2_b = np.ascontiguousarray(
        np.tile(inp["b2"].astype(np.float32)[None, :], (P, 1)))

    counts = [int((mask[b] == 1).sum()) for b in range(B)]
    tkey = T_PAD if max(counts) <= T_PAD else ((max(counts) + P - 1) // P) * P
    nch = tkey // P
    nchp = nch + (nch % 2)
    prog = _get_program(tkey)
    core_ids = list(range(NCORES))
    profile = bool(os.environ.get("KERNEL_PROFILE"))
    kw = {"trace": True} if profile else {}

    # per-batch compacted key tokens + ones-column layout [P, NCHP]
    xk_b, oc_b = [], []
    for b in range(B):
        idx = np.nonzero(mask[b] == 1)[0]
        n = len(idx)
        xk = np.zeros((tkey, D), bf16_np)
        xk[:n] = x[b * S:(b + 1) * S][idx].astype(bf16_np)
        oc = np.zeros((nch * P,), np.float32)
        oc[:n] = 1.0
        ocp = np.zeros((P, nchp), np.float32)
        ocp[:, :nch] = oc.reshape(nch, P).T
        xk_b.append(np.ascontiguousarray(xk))
        oc_b.append(np.ascontiguousarray(ocp))

    in_maps = []
    for c in range(NCORES):
        b = c // 4
        in_maps.append({
            "x": np.ascontiguousarray(x[c * TOK:(c + 1) * TOK, :]),
            "xk": xk_b[b], "oc": oc_b[b],
            "wq": wq_p, "wk": wk_p, "wv": wv_p,
            "bq": bq_p, "bk": bk_p, "bvb": bv_b,
            "wo": wo_p, "bob": bo_b,
            "w1": w1_p, "b1": b1_p, "w2": w2_p, "b2b": b2_b,
        })
    r = run_bass_kernel_spmd(prog, in_maps, core_ids, **kw)

    if profile:
        LAST_EXEC_NS.clear()
        LAST_EXEC_NS["l1"] = 0
        LAST_EXEC_NS["l2"] = r.exec_time_ns
        LAST_EXEC_NS["l2_trace"] = getattr(r, "instructions_and_trace", None)

    out = np.concatenate([r.results[c]["out"] for c in range(NCORES)], axis=0)
    return out.reshape(B, S, D).astype(np.float32)


# revision 27
# speedup vs baseline: 1.1253x; 1.0505x over previous
"""Trainium2 Bass kernel for a dense transformer decoder layer.

Shapes (hardcoded): B=2, S=2048, D=1024, H=16, HD=64, FF=4096, fp32 I/O.

Single fused launch, token-parallel over 8 cores (512 query tokens each;
batch b owned by cores 4b..4b+3). Each core redundantly computes K/V for
its batch's mask-compacted key tokens (host drops mask==0 keys and
zero-pads to a chunk multiple; zero K column -> harmless exp, zero
ones-column/V row -> no ctx or denominator contribution, so no mask bias
is needed anywhere). The ~12us of duplicated K/V GEMM work hides under
the softmax-exp ScalarE critical path and removes the second launch's
ramp/drain plus the K/V HBM round trip.

Precision: fp8e4m3 with DoubleRow matmuls for QKV projections, scores
operands, exp weights, V, and Wo; LN statistics, residual stream and
softmax normalization fp32; FFN in bf16 (fp8 exceeds the error budget).
"""

import os
import numpy as np
import ml_dtypes
from contextlib import ExitStack

import concourse.bass as bass
from concourse import bacc
import concourse.mybir as mybir
import concourse.tile as tile
from concourse.bass_utils import run_bass_kernel_spmd
from concourse.masks import make_identity

B, S, D, H, FF = 2, 2048, 1024, 16, 4096
HD = D // H
EPS = 1e-5
NCORES = 8
TOK = (B * S) // NCORES          # 512 query tokens per core
P = 128
DC = D // P                      # 8 contraction chunks
SBLK = TOK // P                  # 4 s-blocks of 128
FB = FF // P                     # 32 ff blocks of 128

F32 = mybir.dt.float32
BF16 = mybir.dt.bfloat16
FP8 = mybir.dt.float8e4
AF = mybir.ActivationFunctionType
ALU = mybir.AluOpType
DR = mybir.MatmulPerfMode.DoubleRow

bf16_np = ml_dtypes.bfloat16
fp8_np = ml_dtypes.float8_e4m3

T_PAD = 1152  # compacted key columns (only mask==1 keys kept, zero-padded)


def _ln_tile(nc, pools, x_tile, out_tile, eps_sb):
    """LayerNorm (no affine) of one [128, D] fp32 tile into out_tile."""
    stats = pools.tile([P, 2, 6], F32, tag="ln_stats")
    mv = pools.tile([P, 2], F32, tag="ln_mv")
    xg = x_tile.rearrange("p (g d) -> p g d", g=2)
    for g in range(2):
        nc.vector.bn_stats(out=stats[:, g, :], in_=xg[:, g, :])
    nc.vector.bn_aggr(out=mv[:], in_=stats[:])
    mean = mv[:, 0:1]
    std = pools.tile([P, 1], F32, tag="ln_std")
    nc.scalar.activation(out=std, in_=mv[:, 1:2], func=AF.Sqrt, bias=eps_sb, scale=1.0)
    nc.vector.reciprocal(out=std, in_=std)
    nc.vector.tensor_scalar(
        out=out_tile,
        in0=x_tile,
        scalar1=mean,
        scalar2=std,
        op0=ALU.subtract,
        op1=ALU.mult,
    )


def _build_fused(TKEY=T_PAD):
    NCH = TKEY // P                  # real key chunks (9 for 1152)
    NCHP = NCH + (NCH % 2)           # padded to even for DoubleRow (10)
    nc = bacc.Bacc(None, target_bir_lowering=False, debug=False)
    x_d = nc.declare_dram_parameter("x", [TOK, D], F32, isOutput=False)
    xk_d = nc.declare_dram_parameter("xk", [TKEY, D], BF16, isOutput=False)
    oc_d = nc.declare_dram_parameter("oc", [P, NCHP], F32, isOutput=False)
    wq_d = nc.declare_dram_parameter("wq", [P, DC * D], FP8, isOutput=False)
    wk_d = nc.declare_dram_parameter("wk", [P, DC * D], FP8, isOutput=False)
    wv_d = nc.declare_dram_parameter("wv", [P, DC * D], FP8, isOutput=False)
    bq_d = nc.declare_dram_parameter("bq", [P, DC], F32, isOutput=False)
    bk_d = nc.declare_dram_parameter("bk", [P, DC], F32, isOutput=False)
    bv_d = nc.declare_dram_parameter("bvb", [P, D], F32, isOutput=False)
    wo_d = nc.declare_dram_parameter("wo", [P, DC * D], FP8, isOutput=False)
    bo_d = nc.declare_dram_parameter("bob", [P, D], F32, isOutput=False)
    w1_d = nc.declare_dram_parameter("w1", [D, FF], BF16, isOutput=False)
    b1_d = nc.declare_dram_parameter("b1", [P, FB], F32, isOutput=False)
    w2_d = nc.declare_dram_parameter("w2", [P, FB * D], BF16, isOutput=False)
    b2_d = nc.declare_dram_parameter("b2b", [P, D], F32, isOutput=False)
    out_d = nc.declare_dram_parameter("out", [TOK, D], F32, isOutput=True)

    with tile.TileContext(nc) as tc, ExitStack() as ctx:
        glob = ctx.enter_context(tc.tile_pool(name="glob", bufs=1))

        ident = glob.tile([P, P], BF16)
        make_identity(nc, ident)
        eps_sb = glob.tile([P, 1], F32)
        nc.vector.memset(eps_sb, EPS)
        ones_sb = glob.tile([1, 64], BF16)
        nc.vector.memset(ones_sb, 1.0)
        negone_sb = glob.tile([P, 1], F32)
        nc.vector.memset(negone_sb, -1.0)
        onesh_sb = glob.tile([P, H], F32)
        nc.vector.memset(onesh_sb, 1.0)

        qt_sb = glob.tile([P, DC, TOK], FP8)
        kt_sb = glob.tile([P, DC, TKEY], FP8)
        va_sb = glob.tile([P, NCHP, H, HD + 1], FP8)
        ctxu_sb = glob.tile([P, DC, TOK], BF16)      # unnormalized ctx^T
        ctxn_sb = glob.tile([P, DC, TOK], FP8)       # normalized ctx^T
        exp_bufs = [glob.tile([P, NCHP, 2, TOK], FP8, name=f"expb{i}")
                    for i in range(3)]
        if NCHP != NCH:
            for eb in exp_bufs:
                nc.gpsimd.memset(eb[:, NCH:NCHP, :, :], 0.0)
            nc.gpsimd.memset(va_sb[:, NCH:NCHP, :, :], 0.0)

        # residual-stream x tiles (also LN1 input) + misc small loads;
        # key-token tiles go first on the sync queue (critical path)
        xk_tiles = []
        for sb in range(TKEY // P):
            xkt = glob.tile([P, D], BF16, name=f"xk{sb}")
            nc.sync.dma_start(out=xkt, in_=xk_d[sb * P:(sb + 1) * P, :])
            xk_tiles.append(xkt)
        oc_sb = glob.tile([P, NCHP], F32)
        nc.sync.dma_start(out=oc_sb, in_=oc_d[:, :])
        x_tiles = []
        for sb in range(SBLK):
            xt = glob.tile([P, D], F32, name=f"x{sb}")
            nc.sync.dma_start(out=xt, in_=x_d[sb * P:(sb + 1) * P, :])
            x_tiles.append(xt)
        bo_sb = glob.tile([P, D], F32)
        b2_sb = glob.tile([P, D], F32)
        b1_sb = glob.tile([P, FB], F32)
        wo_sb = glob.tile([P, DC, D], FP8)
        resid_sb = glob.tile([P, SBLK, D], F32)
        z2t_sb = glob.tile([P, DC, TOK], BF16)

        with tc.tile_pool(name="qkv", bufs=1) as qkvp, \
             tc.tile_pool(name="lnw", bufs=2) as lnw, \
             tc.tile_pool(name="attn_sc", bufs=3, space="PSUM") as sc_psum, \
             tc.tile_pool(name="attn_cx", bufs=2, space="PSUM") as cx_psum, \
             tc.tile_pool(name="attn_wk", bufs=1) as awork:
            wq_sb = qkvp.tile([P, DC, D], FP8)
            wk_sb = qkvp.tile([P, DC, D], FP8)
            wv_sb = qkvp.tile([P, DC, D], FP8)
            bq_sb = qkvp.tile([P, DC], F32)
            bk_sb = qkvp.tile([P, DC], F32)
            bv_sb = qkvp.tile([P, D], F32)
            for dc in range(DC):
                nc.gpsimd.dma_start(out=wk_sb[:, dc, :],
                                    in_=wk_d[:, dc * D:(dc + 1) * D])
                nc.gpsimd.dma_start(out=wq_sb[:, dc, :],
                                    in_=wq_d[:, dc * D:(dc + 1) * D])
                nc.gpsimd.dma_start(out=wv_sb[:, dc, :],
                                    in_=wv_d[:, dc * D:(dc + 1) * D])
            nc.gpsimd.dma_start(out=bq_sb, in_=bq_d[:, :])
            nc.gpsimd.dma_start(out=bk_sb, in_=bk_d[:, :])
            nc.gpsimd.dma_start(out=bv_sb, in_=bv_d[:])
            nc.gpsimd.dma_start(out=bo_sb, in_=bo_d[:])
            nc.gpsimd.dma_start(out=b2_sb, in_=b2_d[:])
            nc.gpsimd.dma_start(out=b1_sb, in_=b1_d[:])
            nc.gpsimd.dma_start(
                out=wo_sb, in_=wo_d[:].rearrange("p (c n) -> p c n", c=DC))

            # LN1 of key tokens -> z1k^T, then own tokens -> z1^T; the
            # PSUM->SBUF copies round-robin over 3 engines so the DVE
            # (busy with LN stats) isn't the serial bottleneck
            z1kt_sb = qkvp.tile([P, DC, TKEY], FP8)
            z1t_sb = qkvp.tile([P, DC, TOK], FP8)
            cpeng = [nc.vector, nc.scalar, nc.vector]

            def ln_t(src, dst_sb, dst_col, ci):
                z1k = lnw.tile([P, D], BF16, tag="z1k")
                _ln_tile(nc, lnw, src, z1k, eps_sb)
                for dc in range(DC):
                    pt = sc_psum.tile([P, P], BF16, tag="sc")
                    nc.tensor.transpose(pt, z1k[:, dc * P:(dc + 1) * P], ident)
                    eng = cpeng[(ci + dc) % 3]
                    if eng is nc.scalar:
                        eng.copy(out=dst_sb[:, dc, dst_col:dst_col + P], in_=pt)
                    else:
                        eng.tensor_copy(
                            out=dst_sb[:, dc, dst_col:dst_col + P], in_=pt)

            for sb in range(NCH):
                ln_t(xk_tiles[sb], z1kt_sb, sb * P, sb)
            for sb in range(SBLK):
                ln_t(x_tiles[sb], z1t_sb, sb * P, sb)

            den8s = [awork.tile([8, TOK], F32, tag="den8", name=f"den8_{i}")
                     for i in range(2)]

            def kq_pair(pair):
                # kT columns for this pair, in <=512-col spans
                for c0 in range(0, TKEY, 512):
                    cw = min(512, TKEY - c0)
                    pk = cx_psum.tile([P, 512], F32, tag="cx")
                    for i in range(DC // 2):
                        nc.tensor.matmul(
                            pk[:, 0:cw],
                            lhsT=wk_sb[:, 2 * i:2 * i + 2, pair * P:(pair + 1) * P],
                            rhs=z1kt_sb[:, 2 * i:2 * i + 2, c0:c0 + cw],
                            start=(i == 0), stop=(i == DC // 2 - 1),
                            perf_mode=DR,
                        )
                    nc.vector.tensor_scalar_add(
                        out=kt_sb[:, pair, c0:c0 + cw], in0=pk[:, 0:cw],
                        scalar1=bk_sb[:, pair:pair + 1])
                pq = cx_psum.tile([P, 512], F32, tag="cx")
                for i in range(DC // 2):
                    nc.tensor.matmul(
                        pq,
                        lhsT=wq_sb[:, 2 * i:2 * i + 2, pair * P:(pair + 1) * P],
                        rhs=z1t_sb[:, 2 * i:2 * i + 2, :],
                        start=(i == 0), stop=(i == DC // 2 - 1),
                        perf_mode=DR,
                    )
                nc.vector.tensor_scalar_add(
                    out=qt_sb[:, pair, :], in0=pq,
                    scalar1=bq_sb[:, pair:pair + 1])

            def scores_pair(pair):
                et = exp_bufs[pair % 3]
                for tb in range(NCH):
                    ps = sc_psum.tile([P, 2, TOK], F32, tag="sc")
                    for hi in range(2):
                        po = 64 * hi
                        nc.tensor.matmul(
                            ps[:, hi, :],
                            lhsT=kt_sb[po:po + 64, pair, tb * P:(tb + 1) * P],
                            rhs=qt_sb[po:po + 64, pair, :],
                            start=True, stop=True,
                            tile_position=(po, 0),
                        )
                    nc.scalar.activation(
                        out=et[:, tb, :, :], in_=ps, func=AF.Exp,
                        bias=negone_sb, scale=1.0)

            def v_build(sbs):
                for sb in sbs:
                    vv = awork.tile([P, D], BF16, tag="vv", name=f"vv{sb}")
                    for vh in range(2):
                        pv = cx_psum.tile([P, 512], F32, tag="cx")
                        for i in range(DC // 2):
                            nc.tensor.matmul(
                                pv,
                                lhsT=z1kt_sb[:, 2 * i:2 * i + 2,
                                             sb * P:(sb + 1) * P],
                                rhs=wv_sb[:, 2 * i:2 * i + 2,
                                          vh * 512:(vh + 1) * 512],
                                start=(i == 0), stop=(i == DC // 2 - 1),
                                perf_mode=DR,
                            )
                        nc.vector.tensor_add(
                            out=vv[:, vh * 512:(vh + 1) * 512], in0=pv,
                            in1=bv_sb[:, vh * 512:(vh + 1) * 512])
                    nc.vector.tensor_scalar(
                        out=va_sb[:, sb, :, 0:HD],
                        in0=vv.rearrange("p (h k) -> p h k", h=H),
                        scalar1=oc_sb[:, sb:sb + 1], scalar2=None,
                        op0=ALU.mult)
                    nc.vector.tensor_scalar(
                        out=va_sb[:, sb, :, HD:HD + 1],
                        in0=onesh_sb.rearrange("p (h o) -> p h o", o=1),
                        scalar1=oc_sb[:, sb:sb + 1], scalar2=None,
                        op0=ALU.mult)

            def ctx_pair(pair):
                et = exp_bufs[pair % 3]
                dstage = awork.tile([1, 2, TOK], F32, tag="dstage",
                                    name=f"dstage_{pair}")
                for hi in range(2):
                    h = pair * 2 + hi
                    pc = cx_psum.tile([HD + 1, TOK], F32, tag="cx")
                    for tg in range(NCHP // 2):
                        nc.tensor.matmul(
                            pc,
                            lhsT=va_sb[:, 2 * tg:2 * tg + 2, h, :],
                            rhs=et[:, 2 * tg:2 * tg + 2, hi, :],
                            start=(tg == 0), stop=(tg == NCHP // 2 - 1),
                            perf_mode=DR,
                        )
                    po = 64 * hi
                    nc.vector.tensor_copy(
                        out=ctxu_sb[po:po + 64, pair, :], in_=pc[0:HD, :])
                    nc.vector.tensor_copy(
                        out=dstage[:, hi, :], in_=pc[HD:HD + 1, :])
                j = pair % 4
                nc.sync.dma_start(
                    out=den8s[pair // 4][2 * j:2 * j + 2, :], in_=dstage)

            def norm_batch(bi):
                den8 = den8s[bi]
                den8r = awork.tile([8, TOK], F32, tag="den8r")
                dscr = awork.tile([8, TOK], F32, tag="dscr")
                nc.vector.reciprocal_approx_accurate(
                    out=den8r, in_=den8, scratch=dscr)
                rc8 = awork.tile([8, TOK], BF16, tag="rc8")
                nc.vector.tensor_copy(out=rc8, in_=den8r)
                rcflat = awork.tile([1, 8, TOK], BF16, tag="rcflat")
                nc.sync.dma_start(out=rcflat, in_=rc8)
                for j in range(4):
                    pj = 4 * bi + j
                    pb = cx_psum.tile([P, TOK], F32, tag="cx")
                    for hi in range(2):
                        po = 64 * hi
                        nc.tensor.matmul(
                            pb[po:po + 64, :], lhsT=ones_sb,
                            rhs=rcflat[:, 2 * j + hi, :],
                            start=True, stop=True,
                            tile_position=(0, po))
                    nc.vector.tensor_mul(
                        out=ctxn_sb[:, pj, :],
                        in0=ctxu_sb[:, pj, :], in1=pb)

            # software pipeline: scores of pair p+1 are emitted before
            # ctx of pair p so the ScalarE exp stream never starves
            kq_pair(0)
            scores_pair(0)
            for pair in range(1, DC):
                kq_pair(pair)
                scores_pair(pair)
                if pair == 1:
                    v_build(range(0, 5))
                elif pair == 2:
                    v_build(range(5, NCH))
                if pair >= 2:
                    ctx_pair(pair - 2)
                if pair == 7:
                    norm_batch(0)
            ctx_pair(DC - 2)
            ctx_pair(DC - 1)
            norm_batch(1)

        # --- Wo (fp8 DoubleRow) + residual + LN2 ---
        with tc.tile_pool(name="wo_psum", bufs=2, space="PSUM") as wo_psum, \
             tc.tile_pool(name="wo_wk", bufs=3) as wwork:
            for sb in range(SBLK):
                pw = wo_psum.tile([P, D], F32, tag="wo")
                for oh in range(2):
                    for i in range(DC // 2):
                        nc.tensor.matmul(
                            pw[:, oh * 512:(oh + 1) * 512],
                            lhsT=ctxn_sb[:, 2 * i:2 * i + 2, sb * P:(sb + 1) * P],
                            rhs=wo_sb[:, 2 * i:2 * i + 2, oh * 512:(oh + 1) * 512],
                            start=(i == 0), stop=(i == DC // 2 - 1),
                            perf_mode=DR,
                        )
                rs = resid_sb[:, sb, :]
                nc.vector.tensor_add(out=rs, in0=pw, in1=x_tiles[sb])
                nc.gpsimd.tensor_tensor(out=rs, in0=rs, in1=bo_sb, op=ALU.add)
                z2 = wwork.tile([P, D], BF16, tag="z2")
                _ln_tile(nc, wwork, rs, z2, eps_sb)
                for dc in range(DC):
                    pt = wo_psum.tile([P, P], BF16, tag="tp2")
                    nc.tensor.transpose(pt, z2[:, dc * P:(dc + 1) * P], ident)
                    nc.vector.tensor_copy(
                        out=z2t_sb[:, dc, sb * P:(sb + 1) * P], in_=pt)

        # --- FFN (bf16; fp8 fails the error budget) ---
        ffn = ctx.enter_context(tc.tile_pool(name="ffn", bufs=1))
        h1t_sb = ffn.tile([P, FB, TOK], BF16)
        with tc.tile_pool(name="ff_psum", bufs=2, space="PSUM") as fa_psum, \
             tc.tile_pool(name="ffb_psum", bufs=4, space="PSUM") as fb_psum, \
             tc.tile_pool(name="ff_w1", bufs=6) as w1pool, \
             tc.tile_pool(name="ff_w2", bufs=6) as w2pool, \
             tc.tile_pool(name="ff_wk", bufs=4) as fwork:
            po0_tiles = [fb_psum.tile([P, 512], F32, tag="ffb", name=f"po0_{sb}")
                         for sb in range(SBLK)]
            for fb in range(FB):
                w1t = w1pool.tile([P, DC, P], BF16, tag="w1t")
                nc.gpsimd.dma_start(
                    out=w1t,
                    in_=w1_d[:, fb * P:(fb + 1) * P].rearrange(
                        "(c p) f -> p c f", p=P))
                pf = fa_psum.tile([P, TOK], F32, tag="ffa")
                for dc in range(DC):
                    nc.tensor.matmul(
                        pf, lhsT=w1t[:, dc, :], rhs=z2t_sb[:, dc, :],
                        start=(dc == 0), stop=(dc == DC - 1))
                nc.scalar.activation(
                    out=h1t_sb[:, fb, :], in_=pf, func=AF.Relu,
                    bias=b1_sb[:, fb:fb + 1], scale=1.0)
                w2t = w2pool.tile([P, 512], BF16, tag="w2t")
                nc.gpsimd.dma_start(out=w2t, in_=w2_d[:, fb * D:fb * D + 512])
                for sb in range(SBLK):
                    nc.tensor.matmul(
                        po0_tiles[sb], lhsT=h1t_sb[:, fb, sb * P:(sb + 1) * P],
                        rhs=w2t,
                        start=(fb == 0), stop=(fb == FB - 1))
            for sb in range(SBLK):
                ot = fwork.tile([P, 512], F32, tag="out")
                nc.vector.tensor_add(out=ot, in0=po0_tiles[sb],
                                     in1=resid_sb[:, sb, 0:512])
                nc.gpsimd.tensor_tensor(out=ot, in0=ot, in1=b2_sb[:, 0:512],
                                        op=ALU.add)
                nc.sync.dma_start(out=out_d[sb * P:(sb + 1) * P, 0:512], in_=ot)
            po1_tiles = [fb_psum.tile([P, 512], F32, tag="ffb", name=f"po1_{sb}")
                         for sb in range(SBLK)]
            for fb in range(FB):
                w2t = w2pool.tile([P, 512], BF16, tag="w2t")
                nc.gpsimd.dma_start(
                    out=w2t, in_=w2_d[:, fb * D + 512:(fb + 1) * D])
                for sb in range(SBLK):
                    nc.tensor.matmul(
                        po1_tiles[sb], lhsT=h1t_sb[:, fb, sb * P:(sb + 1) * P],
                        rhs=w2t,
                        start=(fb == 0), stop=(fb == FB - 1))
            for sb in range(SBLK):
                ot = fwork.tile([P, 512], F32, tag="out")
                nc.vector.tensor_add(out=ot, in0=po1_tiles[sb],
                                     in1=resid_sb[:, sb, 512:1024])
                nc.gpsimd.tensor_tensor(out=ot, in0=ot, in1=b2_sb[:, 512:1024],
                                        op=ALU.add)
                nc.sync.dma_start(out=out_d[sb * P:(sb + 1) * P, 512:1024], in_=ot)

    return nc


_programs = {}
LAST_EXEC_NS = {}


def _get_program(tkey):
    if ("f", tkey) not in _programs:
        f = _build_fused(tkey)
        f.finalize()
        _programs[("f", tkey)] = f
    return _programs[("f", tkey)]


def kernel(**inputs):
    inp = {k: np.asarray(v) for k, v in inputs.items()}
    x = inp["x"].astype(np.float32).reshape(B * S, D)
    mask = inp["mask"].astype(np.int32)

    # ---- host-side weight prep (layout + LN-affine folding, fp32 math) ----
    scale = np.float32(1.0 / np.sqrt(HD))
    Wq = inp["Wq"].astype(np.float32).transpose(1, 0, 2).reshape(D, D)
    Wk = inp["Wk"].astype(np.float32).transpose(1, 0, 2).reshape(D, D)
    Wv = inp["Wv"].astype(np.float32).transpose(1, 0, 2).reshape(D, D)
    g1 = inp["ln1_g"].astype(np.float32)
    b1n = inp["ln1_b"].astype(np.float32)
    g2 = inp["ln2_g"].astype(np.float32)
    b2n = inp["ln2_b"].astype(np.float32)

    def chunk_part(w):  # [D, D] -> [P, DC*D] with row d = dc*128+p
        return np.ascontiguousarray(
            w.reshape(DC, P, D).transpose(1, 0, 2).reshape(P, DC * D))

    wq_p = chunk_part((g1[:, None] * Wq * scale).astype(fp8_np))
    bq_p = np.ascontiguousarray(
        ((b1n @ Wq) * scale + inp["bq"].astype(np.float32).reshape(-1) * scale)
        .reshape(DC, P).T).astype(np.float32)
    wk_p = chunk_part((g1[:, None] * Wk).astype(fp8_np))
    bk_p = np.ascontiguousarray(
        ((b1n @ Wk) + inp["bk"].astype(np.float32).reshape(-1))
        .reshape(DC, P).T).astype(np.float32)
    bv_p = ((b1n @ Wv) + inp["bv"].astype(np.float32).reshape(-1)).astype(np.float32)
    wv_p = chunk_part((g1[:, None] * Wv).astype(fp8_np))
    bv_b = np.ascontiguousarray(np.tile(bv_p[None, :], (P, 1)))

    wo_p = chunk_part(inp["Wo"].astype(np.float32).astype(fp8_np))
    bo_b = np.ascontiguousarray(
        np.tile(inp["bo"].astype(np.float32)[None, :], (P, 1)))
    w1_p = np.ascontiguousarray(
        (g2[:, None] * inp["W1"].astype(np.float32)).astype(bf16_np))
    b1_p = np.ascontiguousarray(
        ((b2n @ inp["W1"].astype(np.float32)) + inp["b1"].astype(np.float32))
        .reshape(FB, P).T).astype(np.float32)
    w2_p = np.ascontiguousarray(
        inp["W2"].astype(np.float32).astype(bf16_np)
        .reshape(FB, P, D).transpose(1, 0, 2).reshape(P, FB * D))
    b2_b = np.ascontiguousarray(
        np.tile(inp["b2"].astype(np.float32)[None, :], (P, 1)))

    counts = [int((mask[b] == 1).sum()) for b in range(B)]
    tkey = T_PAD if max(counts) <= T_PAD else ((max(counts) + P - 1) // P) * P
    nch = tkey // P
    nchp = nch + (nch % 2)
    prog = _get_program(tkey)
    core_ids = list(range(NCORES))
    profile = bool(os.environ.get("KERNEL_PROFILE"))
    kw = {"trace": True} if profile else {}

    # per-batch compacted key tokens + ones-column layout [P, NCHP]
    xk_b, oc_b = [], []
    for b in range(B):
        idx = np.nonzero(mask[b] == 1)[0]
        n = len(idx)
        xk = np.zeros((tkey, D), bf16_np)
        xk[:n] = x[b * S:(b + 1) * S][idx].astype(bf16_np)
        oc = np.zeros((nch * P,), np.float32)
        oc[:n] = 1.0
        ocp = np.zeros((P, nchp), np.float32)
        ocp[:, :nch] = oc.reshape(nch, P).T
        xk_b.append(np.ascontiguousarray(xk))
        oc_b.append(np.ascontiguousarray(ocp))

    in_maps = []
    for c in range(NCORES):
        b = c // 4
        in_maps.append({
            "x": np.ascontiguousarray(x[c * TOK:(c + 1) * TOK, :]),
            "xk": xk_b[b], "oc": oc_b[b],
            "wq": wq_p, "wk": wk_p, "wv": wv_p,
            "bq": bq_p, "bk": bk_p, "bvb": bv_b,
            "wo": wo_p, "bob": bo_b,
            "w1": w1_p, "b1": b1_p, "w2": w2_p, "b2b": b2_b,
        })
    r = run_bass_kernel_spmd(prog, in_maps, core_ids, **kw)

    if profile:
        LAST_EXEC_NS.clear()
        LAST_EXEC_NS["l1"] = 0
        LAST_EXEC_NS["l2"] = r.exec_time_ns
        LAST_EXEC_NS["l2_trace"] = getattr(r, "instructions_and_trace", None)

    out = np.concatenate([r.results[c]["out"] for c in range(NCORES)], axis=0)
    return out.reshape(B, S, D).astype(np.float32)
